# revision 49
# baseline (speedup 1.0000x reference)
"""Soft-DTW layer (band-limited, gamma=1) as a Bass/Tile kernel on 8 TRN2 cores.

Problem: x [64, 512] f32, protos [32, 64] f32 -> out [64, 32, 1] f32
  out[b, f, 0] = softDTW(C[b,f]) / T, C[b,f][i,j] = (x[b,i]-protos[f,j])^2,
  Sakoe-Chiba band |i/511 - j/63| <= 0.2, out-of-band = BIG.

Algorithm (per (b,f) problem, exp-space):
  E'(i,j) = e^{a*i - D(i,j)} satisfies
    E'(i,j) = G(i,j) * (E'(i-1,j) + E'(i-1,j-1) + e^{-a} * E'(i,j-1))
  with G = e^{a - C} (0 outside band). Sweep columns j=0..63; column j's
  window covers rows [8j-104, 8j+112) (216 rows); within it only
  v in [lo_j, hi_j] is in the band (lo = ceil((114+7j)/63),
  hi = floor((12990+7j)/63)).

  G production is a three-engine pipeline, chunked over column groups and
  overlapped with the DVE column recurrence:
    GPSIMD: memset chunk of G to FILLBIG (out-of-band background), then
            write diff = x - protos on the in-band (j,g,v) ranges
            (per-segment APs; lo/hi are piecewise constant in j)
    ACT:    Square in place over the full chunk (background -> 1e24),
            Exp(A - .) in place (background -> 0  =>  band mask for free)
  Per column the vector (DVE) engine runs three ops over the contiguous
  span [lo_j, 216+hi_j] covering both groups (the inter-group seam has
  G=0 so the scan kills any carry across it):
    w = EA*cprev[v+7] + cprev[v+8]          (shifted add, STT)
    b = (w * ECA) * G_j                     (STT; ECA -> se on rescale)
    ccur = scan(G_j, b)                     (mult/add linear recurrence)
  Every FB columns a per-problem-group power-of-two rescale
  s = 2^{127-E} (E = biased exponent of the window max, extracted with
  integer bit ops -- the DVE reciprocal's iterative divide breaks for
  very large maxes) keeps values in f32 range; the E's are accumulated
  and the 2^... factors are folded back in the final extraction.

Sharding: data-parallel over batch. Core c handles b in [8c, 8c+8); its 256
(b,f) problems sit as 2 groups of 128 partitions:
  partition p, group g -> b = 8c + 4g + p//32, f = p%32.
"""

import numpy as np

import concourse.bass as bass
import concourse.bacc as bacc
import concourse.mybir as mybir
import concourse.tile as tile
from concourse.bass_utils import run_bass_kernel_spmd

T, K = 512, 64
NCORES = 8
L = 216          # column window length
CS = L * 2 + 8   # column buffer: [g0 216 | g1 216 | 8 zero pad]
GCOL = 2 * L     # per-column G stride (both groups)
XPAD = 104 + T + 112          # padded x row length (728)
XBIG = 1.0e4                  # pad value; (XBIG-p)^2 ~ 1e8 -> exp -> 0
FILLBIG = 1.0e12              # out-of-band fill for C before exp
A = 0.75                      # rescale slope per row
FB = 16                       # feedback (renorm) every FB columns
EA = float(np.exp(A))
ECA = float(np.exp(-A))
F32 = mybir.dt.float32
NFB = K // FB - 1             # rescale events (skip last block)

LO = [(114 + 7 * j + 62) // 63 for j in range(K)]       # first in-band v
HI = [(12990 + 7 * j) // 63 for j in range(K)]          # last in-band v

# column chunks for the G pipeline (small head chunks for fast fill).
# For the first NSQ chunks GPSIMD also does the squaring so the ACT
# pipeline fill is one Exp per chunk instead of Square+Exp (ACT ops
# carry a ~1.6us fixed cost).
HEAD = [2, 3, 4]        # head chunk sizes; the rest are 8-column chunks
NSQ = 0        # head chunks whose squaring runs on GPSIMD instead of ACT
NPRE = 2       # head chunks whose FILLBIG memset is pre-hoisted before the DMA wait


def _chunks(head):
    out, j = [], 0
    for n in head:
        out.append((j, n)); j += n
    while j < K:
        n = min(8, K - j)
        out.append((j, n)); j += n
    return out


def _segs(chunks):
    """(chunk_j0, j_start, nj, lo, hi) maximal constant-(lo,hi) runs."""
    segs = []
    for j0, nj in chunks:
        j = j0
        while j < j0 + nj:
            je = j
            while (je + 1 < j0 + nj and LO[je + 1] == LO[j]
                   and HI[je + 1] == HI[j]):
                je += 1
            segs.append((j0, j, je - j + 1, LO[j], HI[j]))
            j = je + 1
    return segs


def _ap(t, offset, dims):
    """Custom free-dim access pattern on tile t: dims = [[step, count], ...]
    (element units), keeping the partition dim."""
    ap = t[:, 0:1].copy()
    ap.ap = ap.ap[:1] + [[int(s), int(n)] for s, n in dims]
    ap.offset = int(offset)
    return ap


def build_nc(head=None, nsq=None, npre=None):
    CHUNKS = _chunks(HEAD if head is None else head)
    SEGS = _segs(CHUNKS)
    _nsq = NSQ if nsq is None else nsq
    _npre = NPRE if npre is None else npre
    nc = bacc.Bacc("TRN2")
    xs = nc.dram_tensor("xs", [8, T], F32, kind="ExternalInput")
    pr = nc.dram_tensor("protos", [32, K], F32, kind="ExternalInput")
    out = nc.dram_tensor("out", [128, 2], F32, kind="ExternalOutput")

    with tile.TileContext(nc) as tc:
        with tc.tile_pool(name="main", bufs=1) as pool:
            BF16 = mybir.dt.bfloat16
            x_all = pool.tile([128, 2 * XPAD], F32)   # padded x per problem/group
            prt = pool.tile([128, K], F32)            # protos row per problem
            # DP tiles in bf16: the b tensor_tensor then runs in the DVE's
            # 2x_1p packed mode (fp32 state inside the scan is unchanged;
            # full-bf16 storage measured 8e-4 max rel err vs the 2e-2 gate).
            G = pool.tile([128, K * GCOL], BF16)      # banded G, layout (j, g, v)
            colA = pool.tile([128, CS], BF16)
            colB = pool.tile([128, CS], BF16)
            w = pool.tile([128, CS], BF16)
            b = pool.tile([128, CS], BF16)
            mxb = pool.tile([128, 2], F32)            # rescale max scratch
            mxs = pool.tile([128, 2], mybir.dt.int32)  # bits of 2^{127-E}
            exb = pool.tile([128, 2 * NFB], mybir.dt.int32)  # E slots (k, g)
            exbf = pool.tile([128, 2 * NFB], F32)     # E slots as floats
            ef = pool.tile([128, 2], F32)
            efe = pool.tile([128, 2], mybir.dt.int32)
            eff = pool.tile([128, 2], F32)
            efm = pool.tile([128, 2], mybir.dt.int32)
            lnmant = pool.tile([128, 2], F32)
            lnef = pool.tile([128, 2], F32)
            lnS = pool.tile([128, 2], F32)
            tt = pool.tile([128, 2], F32)
            osb = pool.tile([128, 2], F32)
            acon = pool.tile([128, 1], F32)           # bias const A for Exp
            scr = pool.tile([128, 8], F32)            # Pool pre-touch scratch

            # ---- init ----
            # memset only the x pad regions: keeps them disjoint from the
            # DMA'd interiors so the DMAs start without a WAW wait.
            nc.vector.memset(x_all[:, 0:104], XBIG)
            nc.vector.memset(x_all[:, 104 + T:XPAD + 104], XBIG)
            nc.vector.memset(x_all[:, XPAD + 104 + T:], XBIG)
            nc.vector.memset(colA[:, :], 0.0)
            nc.vector.memset(colB[:, :], 0.0)
            nc.vector.memset(w[:, :], 0.0)    # seam cells never rewritten
            # virtual-corner seed E'(-1,-1)=e^{-a} at row -1 of column -1
            # (column -1 window starts at row -112; row -1 -> pos 111)
            nc.vector.memset(colA[:, 111:112], ECA)
            nc.vector.memset(colA[:, L + 111:L + 112], ECA)
            nc.vector.memset(acon[:, :], A)

            # protos first: tiny, and the Pool subs need it before any x.
            psrc = pr[:, :].unsqueeze(0).broadcast_to([4, 32, K])
            nc.sync.dma_start(prt[:, :], psrc)
            # x: DRAM [8, 512] -> per-group replicated rows (partition p,
            # group g reads row 4g + p//32). Split into a head transfer
            # (rows < XHEAD, enough for the first G chunks) and a tail so
            # the G pipeline can start before the full x arrives; the DMAs
            # serialize on the SP queue in issue order.
            XHEAD = 230
            for g, rows in ((0, slice(0, 4)), (1, slice(4, 8))):
                base = g * XPAD + 104
                srch = xs[rows, 0:XHEAD].unsqueeze(1).broadcast_to(
                    [4, 32, XHEAD])
                nc.sync.dma_start(x_all[:, base:base + XHEAD], srch)
            for g, rows in ((0, slice(0, 4)), (1, slice(4, 8))):
                base = g * XPAD + 104
                srct = xs[rows, XHEAD:T].unsqueeze(1).broadcast_to(
                    [4, 32, T - XHEAD])
                nc.sync.dma_start(x_all[:, base + XHEAD:base + T], srct)
            # head-chunk G backgrounds can fill while the DMAs are in flight
            for j0, nj in CHUNKS[:_npre]:
                nc.gpsimd.memset(G[:, j0 * GCOL:(j0 + nj) * GCOL], FILLBIG)
            # The HW has very few sem-wait slots per instruction; Pool (the
            # only consumer of x/protos) touches each DMA'd region with a
            # 1-wait copy so its sub ops inherit the deps via program order.
            nc.gpsimd.tensor_copy(scr[:, 0:1], x_all[:, 104:105])
            nc.gpsimd.tensor_copy(scr[:, 1:2], x_all[:, XPAD + 104:XPAD + 105])
            nc.gpsimd.tensor_copy(scr[:, 2:3], prt[:, 0:1])
            tc.no_sync_barrier()

            # last column fully covered by the head transfer (x index
            # 8j + hi_j < 104 + XHEAD)
            JHEAD = max(j for j in range(K) if 8 * j + HI[j] < 104 + XHEAD)
            tail_touched = False

            # ---- G pipeline (GPSIMD + ACT), chunked ----
            seg_i = 0
            for ci, (j0, nj) in enumerate(CHUNKS):
                if not tail_touched and j0 + nj - 1 > JHEAD:
                    # first chunk needing x rows beyond the head transfer
                    nc.gpsimd.tensor_copy(scr[:, 4:5],
                                          x_all[:, 104 + XHEAD:105 + XHEAD])
                    nc.gpsimd.tensor_copy(
                        scr[:, 5:6],
                        x_all[:, XPAD + 104 + XHEAD:XPAD + 105 + XHEAD])
                    tail_touched = True
                gc = G[:, j0 * GCOL:(j0 + nj) * GCOL]
                if ci >= _npre:
                    nc.gpsimd.memset(gc, FILLBIG)
                while seg_i < len(SEGS) and SEGS[seg_i][0] == j0:
                    _, js, njs, lo, hi = SEGS[seg_i]
                    n_in = hi - lo + 1
                    gseg = _ap(G, js * GCOL + lo,
                               [[GCOL, njs], [L, 2], [1, n_in]])
                    xseg = _ap(x_all, 8 * js + lo,
                               [[8, njs], [XPAD, 2], [1, n_in]])
                    pseg = _ap(prt, js, [[1, njs], [0, 2], [0, n_in]])
                    nc.gpsimd.tensor_tensor(gseg, xseg, pseg,
                                            op=mybir.AluOpType.subtract)
                    seg_i += 1
                if ci < _nsq:
                    nc.gpsimd.tensor_tensor(gc, gc, gc,
                                            op=mybir.AluOpType.mult)
                else:
                    nc.scalar.activation(gc, gc,
                                         mybir.ActivationFunctionType.Square)
                nc.scalar.activation(gc, gc,
                                     mybir.ActivationFunctionType.Exp,
                                     bias=acon[:, :], scale=-1.0)

            # ---- column loop (DVE) ----
            fb_pending = False
            fb_k = 0
            cprev, ccur = colA, colB
            for j in range(K):
                s0 = LO[j] & ~1                       # even start: 4B-aligned
                s1 = L + HI[j]                        # inclusive span
                gbase = j * GCOL
                # w = ECA*cprev[v+8] + cprev[v+7]  (ECA pre-folded so b is a
                # plain tensor_tensor, eligible for the bf16 2x mode). Only
                # the per-group in-band ranges: the seam cells it leaves
                # stale are multiplied by G=0 in the b op.
                n_in = HI[j] - LO[j] + 1
                nc.vector.scalar_tensor_tensor(
                    _ap(w, LO[j], [[L, 2], [1, n_in]]),
                    _ap(cprev, LO[j] + 8, [[L, 2], [1, n_in]]), ECA,
                    _ap(cprev, LO[j] + 7, [[L, 2], [1, n_in]]),
                    op0=mybir.AluOpType.mult, op1=mybir.AluOpType.add)
                gcol = G[:, gbase + s0:gbase + s1 + 1]
                if fb_pending:
                    # per-group rescale scalar s = 2^{127-E} (w already
                    # carries the ECA factor); ranges [s0, L-1] and [L, s1]
                    # tile the whole span so the seam is rewritten (b=0 there
                    # via G=0), keeping the scan carry dead across groups.
                    for g, (glo, ghi) in enumerate(((s0, L - 1), (L, s1))):
                        nc.vector.scalar_tensor_tensor(
                            b[:, glo:ghi + 1], w[:, glo:ghi + 1],
                            mxs[:, g:g + 1].bitcast(F32),
                            G[:, gbase + glo:gbase + ghi + 1],
                            op0=mybir.AluOpType.mult,
                            op1=mybir.AluOpType.mult)
                    fb_pending = False
                else:
                    nc.vector.tensor_tensor(
                        b[:, s0:s1 + 1], w[:, s0:s1 + 1], gcol,
                        op=mybir.AluOpType.mult)
                nc.vector.tensor_tensor_scan(
                    ccur[:, s0:s1 + 1], gcol, b[:, s0:s1 + 1], 0.0,
                    op0=mybir.AluOpType.mult, op1=mybir.AluOpType.add)

                if (j + 1) % FB == 0 and j < K - 1:
                    # power-of-two rescale: s = 2^{127-E} with E the biased
                    # exponent of a sampled window max (stride-4 sampling only
                    # shifts the normalization anchor by a bounded factor; the
                    # accounting via exb stays exact). Bit ops instead of the
                    # iterative-divide reciprocal, which breaks for huge maxes.
                    nsamp = (HI[j] - LO[j]) // 4 + 1
                    nc.vector.tensor_reduce(
                        mxb[:, :],
                        _ap(ccur, LO[j], [[L, 2], [4, nsamp]]),
                        axis=mybir.AxisListType.X, op=mybir.AluOpType.max)
                    ee = exb[:, 2 * fb_k:2 * fb_k + 2]
                    nc.vector.tensor_scalar(
                        ee, mxb[:, :].bitcast(mybir.dt.int32), 23, None,
                        op0=mybir.AluOpType.logical_shift_right)
                    # bits of 2^{127-E}: (254 - E) << 23 in one fused op
                    nc.vector.tensor_scalar(
                        mxs[:, :], ee, -(1 << 23), 254 << 23,
                        op0=mybir.AluOpType.mult, op1=mybir.AluOpType.add)
                    fb_k += 1
                    fb_pending = True
                cprev, ccur = ccur, cprev

            last = cprev  # column 63 buffer
            # ---- extraction: D = a*511 - sum(lnmx) - ln(E'fin); out = D/512 ----
            nc.vector.tensor_copy(ef[:, :], _ap(last, 111, [[L, 2]]))
            # ACT's Ln mishandles tiny args (E'fin can be ~1e-37), so do a
            # frexp-style log: ln(ef) = Ln(mantissa) + (exp - 127)*ln2.
            # (the -127*ln2 is folded into the final affine)
            eiv = ef[:, :].bitcast(mybir.dt.int32)
            nc.vector.tensor_scalar(efe[:, :], eiv, 23, None,
                                    op0=mybir.AluOpType.arith_shift_right)
            nc.vector.tensor_copy(eff[:, :], efe[:, :])   # int -> float value
            nc.vector.tensor_scalar(efm[:, :], eiv, 0x007FFFFF, 0x3F800000,
                                    op0=mybir.AluOpType.bitwise_and,
                                    op1=mybir.AluOpType.bitwise_or)
            # ln(mantissa) on [1,2) as a quartic on DVE (max abs err 7e-5;
            # negligible vs the 2e-2 gate) -- keeps the tail off ACT, whose
            # per-op fixed cost is ~1.6us. The a0 term is folded into the
            # final affine. p = (((a4*m + a3)*m + a2)*m + a1)*m [+ a0]
            PA4, PA3, PA2, PA1, PA0 = (-0.05545931374208463, 0.440502738630572,
                                       -1.4551947720667835, 2.806980531443984,
                                       -1.7367597385211486)
            mant = efm[:, :].bitcast(F32)
            nc.vector.tensor_scalar_mul(lnmant[:, :], mant, PA4)
            for ak in (PA3, PA2, PA1):
                nc.vector.scalar_tensor_tensor(
                    lnmant[:, :], lnmant[:, :], ak, mant,
                    op0=mybir.AluOpType.add, op1=mybir.AluOpType.mult)
            # Sum the rescale exponents (each rescale multiplied the values
            # by 2^{127-E_k}, so ln-true accumulates (E_k-127)*ln2; the
            # -127*ln2 constants are folded into the final affine).
            nc.vector.tensor_copy(exbf[:, :], exb[:, :])   # int -> float
            nc.vector.tensor_reduce(
                lnS[:, :], exbf[:, :].rearrange("p (k g) -> p g k", g=2),
                axis=mybir.AxisListType.X, op=mybir.AluOpType.add)
            nc.vector.tensor_tensor(tt[:, :], lnS[:, :], eff[:, :],
                                    op=mybir.AluOpType.add)
            nc.vector.scalar_tensor_tensor(
                lnef[:, :], tt[:, :], float(np.log(2.0)), lnmant[:, :],
                op0=mybir.AluOpType.mult, op1=mybir.AluOpType.add)
            nc.vector.tensor_scalar(
                osb[:, :], lnef[:, :], float(-1.0 / T),
                float((A * (T - 1) + (NFB + 1) * 127.0 * np.log(2.0) - PA0) / T),
                op0=mybir.AluOpType.mult, op1=mybir.AluOpType.add)
            nc.sync.dma_start(out[:, :], osb[:, :])

    nc.compile()
    return nc


_NC = None


def _get_nc():
    global _NC
    if _NC is None:
        _NC = build_nc()
    return _NC


def kernel(x: np.ndarray, protos: np.ndarray) -> np.ndarray:
    x = np.ascontiguousarray(x, dtype=np.float32)
    protos = np.ascontiguousarray(protos, dtype=np.float32)
    nc = _get_nc()
    in_maps = [
        {"xs": x[8 * c: 8 * c + 8], "protos": protos} for c in range(NCORES)
    ]
    res = run_bass_kernel_spmd(nc, in_maps, core_ids=list(range(NCORES)))
    out = np.empty((64, 32, 1), dtype=np.float32)
    for c in range(NCORES):
        r = res.results[c]["out"]                 # [128, 2]
        blk = r.reshape(4, 32, 2).transpose(2, 0, 1)  # [g, bb, f]
        out[8 * c: 8 * c + 8, :, 0] = blk.reshape(8, 32)
    return out


if __name__ == "__main__":
    x = np.load("/root/problem/x.npy")
    protos = np.load("/root/problem/protos.npy")
    got = kernel(x, protos)
    D_true = np.load("/root/problem/D_true.npy").reshape(64, 32) / T
    rel = np.abs(got[:, :, 0] - D_true) / np.abs(D_true)
    print("rel err max", rel.max(), "mean", rel.mean())
